# revision 53
# baseline (speedup 1.0000x reference)
"""Trainium2 Bass kernel for a ViT-style transformer block (pre-norm).

Strategy (v2):
  - Pure data parallelism: 64 batches -> 8 per NeuronCore, no collectives.
  - Activations feature-major on device (xT: [D, tokens]); host transposes.
  - LN affine + biases folded on the host into adjacent weights/biases.
  - Mixed precision tuned to the 2e-2 gate (numpy-sim'd at ~1.8e-3):
      * QKV / out-proj / attnV matmuls in fp8e4m3 with DoubleRow perf mode
        (2 contraction tiles per instruction). Weights pre-scaled x64
        (x16 for Wo) to clear the e4m3 subnormal range; dequant rides the
        exp scale (2^-12), the mask values (2^-6), and the vT ones-column
        (x64) -- zero extra device ops.
      * scores in bf16 (qT/kT), keys zero-padded to 256 so pad rows make
        exp(0)=1 which is cancelled by zeroed vT pad rows.
      * FFN entirely in bf16 (fp8 there costs ~1.7e-2 rel err).
  - Softmax denominator comes free as a 65th vT column (64*mask), so no
    separate sums matmuls; r = 1/denom folds the 2^-4 attnT dequant.
  - LN stats via f32r ones-matmuls (fast path); all LN1 stats hoisted to
    a prologue so group boundaries never stall the PE.
  - All weights resident in SBUF (fp8/bf16 shrink them to ~12MB).
"""

import numpy as np
import ml_dtypes


def to_fp32r(a):
    """Round fp32 -> fp32r (e8m11, round-to-nearest-even), keep fp32 layout."""
    u = np.ascontiguousarray(a, np.float32).view(np.uint32)
    r = (u + np.uint32(0x7FF) + ((u >> np.uint32(12)) & np.uint32(1))) & np.uint32(
        0xFFFFF000
    )
    return r.view(np.float32)

import concourse.bacc as bacc
import concourse.mybir as mybir
from concourse.bass_utils import run_bass_kernel_spmd
from concourse.tile import TileContext

F32 = mybir.dt.float32
F32R = mybir.dt.float32r
BF16 = mybir.dt.bfloat16
FP8 = mybir.dt.float8e4
AF = mybir.ActivationFunctionType
OP = mybir.AluOpType
DR = mybir.MatmulPerfMode.DoubleRow

DEBUG = False

N_CORES = 8
B, S, D, H, FF = 64, 197, 768, 12, 3072
DH = D // H  # 64
EPS = 1e-6
P = 128
CT = D // P  # 6 contraction tiles
FT = FF // P  # 24
KP = 256  # padded key dim for scores lhsT
SB2 = 224  # per-batch padded token pitch in xh (32-aligned DR slices)
ATTNV_DR = True  # DoubleRow for attnV (vT pair stride 768, width 64)
GB = 2  # batches per group
S1 = S - P  # 69
GTP = 416  # attnT column pitch (32-aligned moving pair stride for out-proj)


def _ln_stats(nc, psA, sml, rep, sqpool, xt_g, onesr, N, tag_pfx, eps_sb,
              rep_tags=("a_rep", "b_rep"), rep_bufs=2):
    """LN stats for one [128, CT, N] chunk -> broadcast alpha/beta tiles."""
    ps_sum = psA.tile([1, N], F32, tag="mm", bufs=2, name=f"{tag_pfx}_pssum")
    ps_sq = psA.tile([1, N], F32, tag="mm", bufs=2, name=f"{tag_pfx}_pssq")
    for ct in range(CT):
        sq = sqpool.tile([P, N], F32R, tag="sq", name=f"{tag_pfx}_sq{ct}")
        nc.scalar.activation(sq[:], xt_g[:, ct, :], AF.Square)
        nc.tensor.matmul(
            ps_sum[:], onesr[:, 0:1], xt_g[:, ct, :],
            start=(ct == 0), stop=(ct == CT - 1),
        )
        nc.tensor.matmul(
            ps_sq[:], onesr[:, 0:1], sq[:],
            start=(ct == 0), stop=(ct == CT - 1),
        )
    m = sml.tile([1, N], F32, tag="st_m", name=f"{tag_pfx}_m")
    msq = sml.tile([1, N], F32, tag="st_msq", name=f"{tag_pfx}_msq")
    var = sml.tile([1, N], F32, tag="st_var", name=f"{tag_pfx}_var")
    alpha = sml.tile([1, N], F32, tag="st_al", name=f"{tag_pfx}_al")
    ascr = sml.tile([1, N], F32, tag="st_ascr", name=f"{tag_pfx}_ascr")
    nc.vector.tensor_scalar_mul(m[:], ps_sum[:], 1.0 / D)
    nc.vector.tensor_scalar_mul(msq[:], ps_sq[:], 1.0 / D)
    nc.vector.tensor_mul(var[:], m[:], m[:])
    nc.vector.tensor_sub(var[:], msq[:], var[:])
    nc.scalar.activation(var[:], var[:], AF.Sqrt, bias=eps_sb[0:1, 0:1], scale=1.0)
    nc.vector.reciprocal_approx_accurate(
        out=alpha[0:1, :], in_=var[0:1, :], scratch=ascr[0:1, :]
    )
    alpha_h = sml.tile([1, N], BF16, tag="st_alh", name=f"{tag_pfx}_alh")
    beta_h = sml.tile([1, N], BF16, tag="st_beh", name=f"{tag_pfx}_beh")
    nc.vector.tensor_scalar_mul(alpha_h[:], alpha[:], 1.0)
    nc.vector.scalar_tensor_tensor(
        beta_h[:], m[:], -1.0, alpha[:], op0=OP.mult, op1=OP.mult
    )
    a_rep = rep.tile([P, N], BF16, tag=rep_tags[0], bufs=rep_bufs, name=f"{tag_pfx}_arep")
    b_rep = rep.tile([P, N], BF16, tag=rep_tags[1], bufs=rep_bufs, name=f"{tag_pfx}_brep")
    nc.gpsimd.partition_broadcast(a_rep[:], alpha_h[0:1, :])
    nc.gpsimd.partition_broadcast(b_rep[:], beta_h[0:1, :])
    return a_rep, b_rep


def build_nc(n_cores=N_CORES, b_shard=8):
    NG = b_shard // GB  # groups
    T = b_shard * S
    GT = GB * S  # 394

    nc = bacc.Bacc(
        "TRN2", target_bir_lowering=False, debug=False, num_devices=n_cores
    )

    xt_d = nc.dram_tensor("xt", [D, T], F32R, kind="ExternalInput")
    wq_d = nc.dram_tensor("wq", [D, D], FP8, kind="ExternalInput")
    wk_d = nc.dram_tensor("wk", [D, D], FP8, kind="ExternalInput")
    wv_d = nc.dram_tensor("wv", [D, D], FP8, kind="ExternalInput")
    wo_d = nc.dram_tensor("wo", [D, D], FP8, kind="ExternalInput")
    w1_d = nc.dram_tensor("w1", [D, FF], BF16, kind="ExternalInput")
    w2_d = nc.dram_tensor("w2", [FF, D], BF16, kind="ExternalInput")
    bq_d = nc.dram_tensor("bq", [D], F32, kind="ExternalInput")
    bk_d = nc.dram_tensor("bk", [D], F32, kind="ExternalInput")
    bo_d = nc.dram_tensor("bo", [D], F32, kind="ExternalInput")
    b1_d = nc.dram_tensor("b1", [FF], F32, kind="ExternalInput")
    b2_d = nc.dram_tensor("b2", [D], F32, kind="ExternalInput")
    mk_d = nc.dram_tensor("mk", [P, 2 * b_shard], F32, kind="ExternalInput")
    yt_d = nc.dram_tensor("yt", [D, T], F32R, kind="ExternalOutput")
    if DEBUG:
        dbgq_d = nc.dram_tensor("dbgq", [P, CT, GB, S], BF16, kind="ExternalOutput")
        dbgk_d = nc.dram_tensor("dbgk", [P, CT, GB, KP], BF16, kind="ExternalOutput")
        dbgv_d = nc.dram_tensor("dbgv", [P, GB, 2, H, DH], FP8, kind="ExternalOutput")
        dbge_d = nc.dram_tensor("dbge", [P, 2, S], FP8, kind="ExternalOutput")
        dbgr_d = nc.dram_tensor("dbgr", [DH, S], F32, kind="ExternalOutput")
        dbga_d = nc.dram_tensor("dbga", [P, CT, GT], FP8, kind="ExternalOutput")
        dbgx_d = nc.dram_tensor("dbgx", [P, CT, GB, SB2], FP8, kind="ExternalOutput")

    def pon(ap_1d):  # [(o p)] -> [p, o]
        return ap_1d.rearrange("(o p) -> p o", p=P)

    def ponn(ap_2d):  # [(o p), n] -> [p, o, n]
        return ap_2d.rearrange("(o p) n -> p o n", p=P)

    with TileContext(nc) as tc:
        with (
            tc.tile_pool(name="const", bufs=1) as const,
            tc.tile_pool(name="xres", bufs=1) as xres,
            tc.tile_pool(name="sml", bufs=1) as sml,
            tc.tile_pool(name="rep", bufs=2) as rep,
            tc.tile_pool(name="sqp", bufs=2) as sqpool,
            tc.tile_pool(name="scr", bufs=2) as scr,
        ):
            wbuf = const.tile([P, 4 * CT, D], FP8, tag="wbuf", name="wbuf")
            w1b = const.tile([P, CT, FF], BF16, tag="w1b", name="w1b")

            bq_sb = const.tile([P, CT], F32, tag="bq", name="bq_sb")
            bk_sb = const.tile([P, CT], F32, tag="bk", name="bk_sb")
            bo_sb = const.tile([P, CT], F32, tag="bo", name="bo_sb")
            b2_sb = const.tile([P, CT], F32, tag="b2", name="b2_sb")
            b1_sb = const.tile([P, FT], F32, tag="b1", name="b1_sb")
            mk_sb = const.tile([P, 2 * b_shard], F32, tag="mk", name="mk_sb")
            mk16 = const.tile([P, 2 * b_shard], FP8, tag="mk16", name="mk16_sb")
            ones = const.tile([P, 1], F32, tag="ones", name="ones_sb")
            onesr = const.tile([P, 1], F32R, tag="onesr", name="onesr_sb")
            eps_sb = const.tile([P, 1], F32, tag="eps", name="eps_sb")
            nc.vector.memset(eps_sb[:], EPS)
            nc.vector.memset(ones[:], 1.0)
            nc.vector.tensor_scalar_mul(onesr[:], ones[:], 1.0)

            # x first (LN1 prologue needs it), split per ct for queue overlap
            xt_g = []
            for g in range(NG):
                xg = xres.tile([P, CT, GT], F32R, tag=f"xt{g}", name=f"xt{g}")
                for ct in range(CT):
                    nc.sync.dma_start(
                        out=xg[:, ct, :],
                        in_=ponn(xt_d[:])[:, ct, g * GT : (g + 1) * GT],
                    )
                xt_g.append(xg)
            nc.sync.dma_start(out=wbuf[:, 0:CT, :], in_=ponn(wq_d[:]))
            nc.sync.dma_start(out=wbuf[:, CT : 2 * CT, :], in_=ponn(wk_d[:]))
            nc.sync.dma_start(out=wbuf[:, 2 * CT : 3 * CT, :], in_=ponn(wv_d[:]))
            nc.sync.dma_start(out=bq_sb[:], in_=pon(bq_d[:]))
            nc.sync.dma_start(out=bk_sb[:], in_=pon(bk_d[:]))
            nc.sync.dma_start(out=mk_sb[:], in_=mk_d[:])
            nc.vector.tensor_scalar_mul(mk16[:], mk_sb[:], 1024.0)
            nc.sync.dma_start(out=wbuf[:, 3 * CT : 4 * CT, :], in_=ponn(wo_d[:]))
            nc.sync.dma_start(out=bo_sb[:], in_=pon(bo_d[:]))
            nc.sync.dma_start(out=b1_sb[:], in_=pon(b1_d[:]))
            nc.sync.dma_start(out=b2_sb[:], in_=pon(b2_d[:]))
            for wc in range(CT):
                nc.sync.dma_start(
                    out=w1b[:, wc : wc + 1, :], in_=ponn(w1_d[:])[:, wc : wc + 1, :]
                )

            ln2_reps = []
            with (
                tc.tile_pool(name="psA", bufs=1, space="PSUM") as psA,
                tc.tile_pool(name="psB", bufs=1, space="PSUM") as psB,
                tc.tile_pool(name="psC", bufs=1, space="PSUM") as psC,
                tc.tile_pool(name="attw", bufs=1) as attw,
                tc.tile_pool(name="attx", bufs=3) as attx,
            ):
                # LN1 + xhat (fp8) for one group; issued one group ahead
                xh_g = {}

                def prep_ln1(g):
                    a_rep, b_rep = _ln_stats(
                        nc, psA, sml, rep, sqpool, xt_g[g], onesr, GT,
                        f"ln1g{g}", eps_sb,
                    )
                    xh = attw.tile([P, CT, GB, SB2], FP8, tag=f"xh{g}", bufs=1,
                                   name=f"xh{g}")
                    for ct in range(CT):
                        t = scr.tile([P, GT], F32, tag="xsc", name=f"xs{g}_{ct}")
                        nc.vector.tensor_mul(t[:], xt_g[g][:, ct, :], a_rep[:])
                        nc.vector.tensor_add(
                            xh[:, ct, :, 0:S],
                            t[:].rearrange("p (b s) -> p b s", b=GB),
                            b_rep[:].rearrange("p (b s) -> p b s", b=GB),
                        )
                    xh_g[g] = xh

                prep_ln1(0)
                for g in range(NG):
                    xg, xh = xt_g[g], xh_g[g]
                    # ---- Q/K projections (fp8 DoubleRow) ----
                    qT = attw.tile([P, CT, GB, S], BF16, tag="qT", bufs=2,
                                   name=f"qT{g}")
                    kT = attw.tile([P, CT, GB, KP], BF16, tag="kT", bufs=2,
                                   name=f"kT{g}")
                    nc.vector.memset(kT[:, :, :, S:KP], 0.0)
                    for dst, wofs, bias in (
                        (qT, 0, bq_sb), (kT, CT, bk_sb)
                    ):
                        for mt in range(CT):
                            ps = psA.tile([P, GB, SB2], F32, tag="mm", bufs=2,
                                          name=f"psqk{g}_{wofs}_{mt}")
                            for j in range(CT // 2):
                                nc.tensor.matmul(
                                    ps[:],
                                    wbuf[:, wofs + 2 * j : wofs + 2 * j + 2,
                                         mt * P : (mt + 1) * P],
                                    xh[:, 2 * j : 2 * j + 2, :, :],
                                    start=(j == 0), stop=(j == CT // 2 - 1),
                                    perf_mode=DR,
                                )
                            nc.vector.tensor_scalar_add(
                                dst[:, mt, :, 0:S],
                                ps[:, :, 0:S],
                                bias[:, mt : mt + 1],
                            )

                    # ---- V projection (fp8 DR), token-major, mask-scaled ----
                    vT = attw.tile([P, GB, 2, H, DH], FP8, tag="vT", bufs=2,
                                   name=f"vT{g}")
                    # zero pad-key rows (69:128) of the second key tile; rows
                    # 64:69 are rewritten by the real V writes below (memset
                    # partition base must be 32-aligned, so start at 64)
                    nc.vector.memset(vT[DH:P, :, 1, :, :], 0.0)
                    for b2 in range(GB):
                        for tt in range(2):
                            off = tt * P
                            M = P if tt == 0 else S1
                            mi = (g * GB + b2) * 2 + tt
                            for hf in range(2):
                                ps = psA.tile([P, D // 2], F32, tag="mm", bufs=2,
                                              name=f"psv{g}_{b2}_{tt}_{hf}")
                                for j in range(CT // 2):
                                    nc.tensor.matmul(
                                        ps[:M, :],
                                        xh[:, 2 * j : 2 * j + 2, b2, off : off + M],
                                        wbuf[:, 2 * CT + 2 * j : 2 * CT + 2 * j + 2,
                                             hf * (D // 2) : (hf + 1) * (D // 2)],
                                        start=(j == 0), stop=(j == CT // 2 - 1),
                                        perf_mode=DR,
                                    )
                                nc.vector.tensor_scalar_mul(
                                    vT[0:M, b2, tt,
                                       hf * (H // 2) : (hf + 1) * (H // 2), :],
                                    ps[0:M, :].rearrange("p (h d) -> p h d", h=H // 2),
                                    mk_sb[0:M, mi : mi + 1],
                                )

                    # next group's LN1 chain overlaps this group's attention
                    if g + 1 < NG:
                        prep_ln1(g + 1)

                    # ---- attention ----
                    attnT = attw.tile([P, CT, GTP], FP8, tag="attnT", bufs=1,
                                      name=f"at{g}")
                    expA = attx.tile([P, GB, H, 2, SB2], FP8, tag="exp", bufs=1,
                                     name=f"e_{g}")
                    for hp2 in range(H // 2):
                        for b2 in range(GB):
                            mi = (g * GB + b2) * 2
                            for h in (2 * hp2, 2 * hp2 + 1):
                                hp, rh = h // 2, (h % 2) * DH
                                ps_sc = psB.tile([P, 2, KP], F32, tag="sc", bufs=3,
                                                 name=f"s_{g}{b2}{h}")
                                for tt in range(2):
                                    nc.tensor.matmul(
                                        ps_sc[:, tt, 0:S],
                                        kT[rh : rh + DH, hp, b2, tt * P : (tt + 1) * P],
                                        qT[rh : rh + DH, hp, b2, :],
                                        start=True, stop=True,
                                    )
                                nc.scalar.activation(
                                    expA[:, b2, h, :, 0:S], ps_sc[:, :, 0:S],
                                    AF.Exp, scale=2.0 ** -12,
                                )
                            # batched softmax denominators for the head pair
                            ps_s = psB.tile([1, 2, S], F32, tag="sc", bufs=3,
                                            name=f"ss_{g}{b2}{hp2}")
                            nc.tensor.matmul(
                                ps_s[:, :, :],
                                mk16[0:P, mi : mi + 1],
                                expA[:, b2, 2 * hp2 : 2 * hp2 + 2, 0, 0:S],
                                start=True, stop=False,
                            )
                            nc.tensor.matmul(
                                ps_s[:, :, :],
                                mk16[0:S1, mi + 1 : mi + 2],
                                expA[0:S1, b2, 2 * hp2 : 2 * hp2 + 2, 1, 0:S],
                                start=False, stop=True,
                            )
                            rcp = attx.tile([1, 2, S], F32, tag="rcp", bufs=3,
                                            name=f"rc_{g}{b2}{hp2}")
                            rcs = attx.tile([1, 2, S], F32, tag="rcs", bufs=3,
                                            name=f"rv_{g}{b2}{hp2}")
                            nc.vector.reciprocal_approx_accurate(
                                out=rcp[0:1, :, :], in_=ps_s[0:1, :, :],
                                scratch=rcs[0:1, :, :],
                            )
                            for h in (2 * hp2, 2 * hp2 + 1):
                                hp, rh = h // 2, (h % 2) * DH
                                ps_a = psC.tile([DH, KP], F32, tag="pa", bufs=3,
                                                name=f"a_{g}{b2}{h}")
                                if ATTNV_DR:
                                    nc.tensor.matmul(
                                        ps_a[:, 0:SB2],
                                        vT[:, b2, :, h, :],
                                        expA[:, b2, h, :, :],
                                        start=True, stop=True,
                                        perf_mode=DR,
                                    )
                                else:
                                    for tt in range(2):
                                        nc.tensor.matmul(
                                            ps_a[:, 0:S],
                                            vT[:, b2, tt, h, :],
                                            expA[:, b2, h, tt, 0:S],
                                            start=(tt == 0), stop=(tt == 1),
                                        )
                                r_rep = attx.tile([DH, S], F32, tag="rrep",
                                                  name=f"rr_{g}{b2}{h}")
                                nc.gpsimd.partition_broadcast(
                                    r_rep[:], rcp[0:1, h % 2, :]
                                )
                                nc.vector.tensor_mul(
                                    attnT[rh : rh + DH, hp, b2 * S : (b2 + 1) * S],
                                    ps_a[0:DH, 0:S],
                                    r_rep[:],
                                )
                                if DEBUG and g == 0 and b2 == 0 and h == 0:
                                    nc.sync.dma_start(
                                        out=dbge_d[:], in_=expA[:, 0, 0, :, 0:S]
                                    )
                                    nc.sync.dma_start(out=dbgr_d[:], in_=r_rep[:])

                    if DEBUG and g == 0:
                        nc.sync.dma_start(out=dbgq_d[:], in_=qT[:])
                        nc.sync.dma_start(out=dbgk_d[:], in_=kT[:])
                        nc.sync.dma_start(out=dbgv_d[:], in_=vT[:])
                        nc.sync.dma_start(out=dbga_d[:], in_=attnT[:, :, 0:GT])
                        nc.sync.dma_start(out=dbgx_d[:], in_=xh[:])

                    # ---- out-projection (fp8 DR) + residual ----
                    for mt in range(CT):
                        ps = psA.tile([P, GTP], F32, tag="mm", bufs=2,
                                      name=f"pso{g}_{mt}")
                        for j in range(CT // 2):
                            nc.tensor.matmul(
                                ps[:],
                                wbuf[:, 3 * CT + 2 * j : 3 * CT + 2 * j + 2,
                                     mt * P : (mt + 1) * P],
                                attnT[:, 2 * j : 2 * j + 2, :],
                                start=(j == 0), stop=(j == CT // 2 - 1),
                                perf_mode=DR,
                            )
                        nc.vector.scalar_tensor_tensor(
                            xg[:, mt, :], ps[:, 0:GT], bo_sb[:, mt : mt + 1],
                            xg[:, mt, :], op0=OP.add, op1=OP.add,
                        )

                    ln2_reps.append(
                        _ln_stats(
                            nc, psA, sml, rep, sqpool, xg, onesr, GT,
                            f"ln2c{g}", eps_sb,
                            rep_tags=("a2_rep", "b2_rep"), rep_bufs=NG,
                        )
                    )

            # ---------------- Phase B: FFN (bf16) ----------------
            with (
                tc.tile_pool(name="psU", bufs=1, space="PSUM") as psU,
                tc.tile_pool(name="psY", bufs=1, space="PSUM") as psY,
                tc.tile_pool(name="ffw", bufs=1) as ffw,
            ):
                for c in range(NG):
                    xg = xt_g[c]
                    a_rep, b_rep = ln2_reps[c]
                    xh = ffw.tile([P, CT, GT], BF16, tag="xh2", bufs=2,
                                  name=f"xh2_{c}")
                    for ct in range(CT):
                        t = scr.tile([P, GT], F32, tag="xsc", name=f"x2s{c}_{ct}")
                        nc.vector.tensor_mul(t[:], xg[:, ct, :], a_rep[:])
                        nc.vector.tensor_add(xh[:, ct, :], t[:], b_rep[:])

                    ps_y = [
                        psY.tile([P, GT], F32, tag=f"y{mt}", name=f"psy{c}_{mt}")
                        for mt in range(CT)
                    ]
                    for ft in range(FT):
                        w2t = ffw.tile([P, D], BF16, tag="w2s", bufs=3,
                                       name=f"w2_{c}_{ft}")
                        nc.sync.dma_start(out=w2t[:], in_=ponn(w2_d[:])[:, ft, :])
                        ps_u = psU.tile([P, GT], F32, tag="st_sum", bufs=2,
                                        name=f"psu{c}_{ft}")
                        for ct in range(CT):
                            nc.tensor.matmul(
                                ps_u[:],
                                w1b[:, ct, ft * P : (ft + 1) * P],
                                xh[:, ct, :],
                                start=(ct == 0), stop=(ct == CT - 1),
                            )
                        g_sb = ffw.tile([P, GT], BF16, tag="g", bufs=3,
                                        name=f"g{c}_{ft}")
                        nc.scalar.activation(
                            g_sb[:], ps_u[:], AF.Gelu,
                            bias=b1_sb[:, ft : ft + 1], scale=1.0,
                        )
                        for mt in range(CT):
                            nc.tensor.matmul(
                                ps_y[mt][:],
                                w2t[:, mt * P : (mt + 1) * P],
                                g_sb[:],
                                start=(ft == 0), stop=(ft == FT - 1),
                            )
                    for mt in range(CT):
                        nc.vector.scalar_tensor_tensor(
                            xg[:, mt, :], ps_y[mt][:], b2_sb[:, mt : mt + 1],
                            xg[:, mt, :], op0=OP.add, op1=OP.add,
                        )
                        nc.sync.dma_start(
                            out=ponn(yt_d[:])[:, mt, c * GT : (c + 1) * GT],
                            in_=xg[:, mt, :],
                        )

    nc.compile()
    return nc


def host_prep(inputs, b_shard=8):
    """Fold LN affine + biases into weights; build per-core input maps."""
    f = np.float32
    E4 = ml_dtypes.float8_e4m3
    BF = ml_dtypes.bfloat16
    x = np.ascontiguousarray(inputs["x"], dtype=f)
    Wq, bq = np.asarray(inputs["Wq"], f), np.asarray(inputs["bq"], f)
    Wk, bk = np.asarray(inputs["Wk"], f), np.asarray(inputs["bk"], f)
    Wv, bv = np.asarray(inputs["Wv"], f), np.asarray(inputs["bv"], f)
    Wo, bo = np.asarray(inputs["Wo"], f), np.asarray(inputs["bo"], f)
    W1, b1 = np.asarray(inputs["W1"], f), np.asarray(inputs["b1"], f)
    W2, b2 = np.asarray(inputs["W2"], f), np.asarray(inputs["b2"], f)
    ln1w, ln1b = np.asarray(inputs["ln1_w"], f), np.asarray(inputs["ln1_b"], f)
    ln2w, ln2b = np.asarray(inputs["ln2_w"], f), np.asarray(inputs["ln2_b"], f)
    mask = np.asarray(inputs["mask"])

    s = f(1.0 / np.sqrt(DH))
    wq8 = np.asarray((ln1w[:, None] * Wq) * (s * 64.0), E4)
    wk8 = np.asarray((ln1w[:, None] * Wk) * 64.0, E4)
    wv8 = np.asarray((ln1w[:, None] * Wv) * 64.0, E4)
    wo8 = np.asarray(Wo * 16.0, E4)
    w1bf = np.asarray(ln2w[:, None] * W1, BF)
    w2bf = np.asarray(W2, BF)
    bq_e = (ln1b @ Wq + bq) * (s * 64.0)
    bk_e = (ln1b @ Wk + bk) * 64.0
    bv_e = ln1b @ Wv + bv
    bo_e = bv_e @ Wo + bo
    b1_e = ln2b @ W1 + b1

    mask_f = mask.astype(f)

    n_cores = B // b_shard
    in_maps = []
    for c in range(n_cores):
        xs = x[c * b_shard : (c + 1) * b_shard]
        xt = to_fp32r(xs.transpose(2, 0, 1).reshape(D, b_shard * S))
        mk = np.zeros((P, 2 * b_shard), f)
        ms = mask_f[c * b_shard : (c + 1) * b_shard] * f(2.0 ** -6)
        for b_ in range(b_shard):
            mk[:, 2 * b_] = ms[b_, 0:P]
            mk[0 : S - P, 2 * b_ + 1] = ms[b_, P:S]
        in_maps.append(
            {
                "xt": xt,
                "wq": wq8, "wk": wk8, "wv": wv8, "wo": wo8,
                "w1": w1bf, "w2": w2bf,
                "bq": bq_e.astype(f), "bk": bk_e.astype(f),
                "bo": bo_e.astype(f), "b1": b1_e.astype(f),
                "b2": b2.astype(f), "mk": mk,
            }
        )
    return in_maps


_NC_CACHE = {}


def get_nc(n_cores=N_CORES, b_shard=8):
    key = (n_cores, b_shard)
    if key not in _NC_CACHE:
        _NC_CACHE[key] = build_nc(n_cores, b_shard)
    return _NC_CACHE[key]


def kernel(**inputs):
    b_shard = B // N_CORES
    nc = get_nc(N_CORES, b_shard)
    in_maps = host_prep(inputs, b_shard)
    res = run_bass_kernel_spmd(nc, in_maps, list(range(N_CORES)))
    outs = []
    for c in range(N_CORES):
        yt = res.results[c]["yt"]  # [D, b_shard*S]
        outs.append(yt.reshape(D, b_shard, S).transpose(1, 2, 0))
    return np.ascontiguousarray(np.concatenate(outs, axis=0), dtype=np.float32)
